# revision 3
# baseline (speedup 1.0000x reference)
"""Trainium2 Bass kernel for Mixtral-style attention (GQA + NeoX RoPE + causal).

Tensor-parallel over heads across 8 NeuronCores: each core owns 4 query heads
and their shared KV head (GQA group intact). Wqkv is column-sharded, Wo is
row-sharded; per-core fp32 partial outputs are summed on the host.

Per-core dataflow (feature-major layouts; all matmuls bf16 with fp32 PSUM):
  1. QKV projection: qkvT[j, t] = sum_h Wqkv[h, j] * hidden[t, h]
     (stationary = Wqkv tile, moving = hiddenT tile streamed from DRAM)
  2. NeoX RoPE applied to q/k straight out of PSUM (DVE), keeping [d, t] layout
  3. Attention per head with transposed scores: sT[s, t] = k . q so softmax's
     sum runs over the partition dim via a ones-vector matmul; exp on ACT
     without max subtraction (scores are small for this problem size); causal
     masking by multiplying diagonal-block tiles with precomputed 0/1 masks
  4. PV: attnT[d, t] += v[s, d].T @ P[s, t] with v transposed once via PE
  5. Normalization by 1/den replicated across partitions with a K=1 matmul
  6. Output projection row-shard: outT_partial[o, t] accumulated over the
     core's 512 features, written fp32 to DRAM.

SBUF/PSUM pools are phase-scoped (the Tile allocator reserves pool space
statically for the pool's lifetime).
"""

import numpy as np
import ml_dtypes
from contextlib import ExitStack

import concourse.bass as bass
import concourse.tile as tile
from concourse import bacc, mybir
from concourse.bass import ts
from concourse.bass_utils import run_bass_kernel_spmd
from concourse.masks import make_identity

BF16 = mybir.dt.bfloat16
F32 = mybir.dt.float32
AF = mybir.ActivationFunctionType

T = 2048
HID = 4096
NH = 32
NKV = 8
D = 128
NCORES = 8
NHL = NH // NCORES            # 4 query heads per core
HO = HID // 128               # 32 hidden-dim k-tiles
TCH = 512
NTC = T // TCH                # 4 t-chunks
NSB = T // 128                # 16 s-blocks
SCALING = float(D) ** -0.5
ROPE_THETA = 1000000.0


def build_kernel():
    nc = bacc.Bacc("TRN2", target_bir_lowering=False, debug=False, num_devices=NCORES)

    hT = nc.dram_tensor("hT", [HID, T], BF16, kind="ExternalInput")
    wqkv = nc.dram_tensor("wqkv", [6, 128, HO, 128], BF16, kind="ExternalInput")
    wo = nc.dram_tensor("wo", [128, NHL, HID], BF16, kind="ExternalInput")
    cosT = nc.dram_tensor("cosT", [64, T], F32, kind="ExternalInput")
    sinT = nc.dram_tensor("sinT", [64, T], F32, kind="ExternalInput")
    masks = nc.dram_tensor("masks", [128, NTC, TCH], BF16, kind="ExternalInput")
    outT = nc.dram_tensor("outT", [HID, T], F32, kind="ExternalOutput")

    with tile.TileContext(nc) as tc, ExitStack() as ctx:
        # ---- global pools (live across all phases) ----
        per = ctx.enter_context(tc.tile_pool(name="per", bufs=1))
        qT_t = per.tile([128, NHL, T], BF16, tag="qT")
        kT_t = per.tile([128, T], BF16, tag="kT")
        vnat_t = per.tile([128, NSB, 128], BF16, tag="vnat")
        ones_s = per.tile([128, 1], BF16, tag="ones_s")
        ones_r = per.tile([1, 128], F32, tag="ones_r")
        nc.vector.memset(ones_s, 1.0)
        nc.vector.memset(ones_r, 1.0)

        # ================= phase 1: QKV projection =================
        with ExitStack() as p1:
            p1per = p1.enter_context(tc.tile_pool(name="p1per", bufs=1))
            hpool = p1.enter_context(tc.tile_pool(name="hpool", bufs=HO))
            wqpool = p1.enter_context(tc.tile_pool(name="wqpool", bufs=2))
            ropetmp = p1.enter_context(tc.tile_pool(name="ropetmp", bufs=2))
            qkv_ps = p1.enter_context(tc.tile_pool(name="qkv_ps", bufs=2, space="PSUM"))
            tp_ps = p1.enter_context(tc.tile_pool(name="tp_ps", bufs=2, space="PSUM"))

            cos_t = p1per.tile([64, T], F32, tag="cos")
            sin_t = p1per.tile([64, T], F32, tag="sin")
            vT_t = p1per.tile([128, T], BF16, tag="vT")
            ident_t = p1per.tile([128, 128], BF16, tag="ident")
            nc.sync.dma_start(out=cos_t, in_=cosT.ap())
            nc.sync.dma_start(out=sin_t, in_=sinT.ap())
            make_identity(nc, ident_t)

            ht = []
            hT_r = hT.ap().rearrange("(ho p) t -> p ho t", p=128)
            for ho in range(HO):
                t_ = hpool.tile([128, T], BF16, tag="ht")
                nc.sync.dma_start(out=t_, in_=hT_r[:, ho, :])
                ht.append(t_)

            # j blocks: 0-3 = q heads, 4 = k, 5 = v
            for j in (4, 5, 0, 1, 2, 3):
                wq_sb = wqpool.tile([128, HO, 128], BF16, tag="wq")
                nc.sync.dma_start(out=wq_sb, in_=wqkv.ap()[j])
                for cp in range(2):  # halves of the t range
                    ps = qkv_ps.tile([128, 2, TCH], F32, tag="qkvps")
                    for ho in range(HO):
                        for c2 in range(2):
                            c = cp * 2 + c2
                            nc.tensor.matmul(
                                ps[:, c2, :], wq_sb[:, ho, :], ht[ho][:, ts(c, TCH)],
                                start=(ho == 0), stop=(ho == HO - 1),
                            )
                    if j == 5:
                        nc.scalar.copy(
                            out=vT_t[:, ts(cp, 2 * TCH)],
                            in_=ps.rearrange("p a b -> p (a b)"),
                        )
                    else:
                        if j == 4:
                            dst = kT_t
                        else:
                            dst = qT_t[:, j, :]
                        for c2 in range(2):
                            c = cp * 2 + c2
                            x1 = ps[0:64, c2, :]
                            x2 = ps[64:128, c2, :]
                            co = cos_t[:, ts(c, TCH)]
                            si = sin_t[:, ts(c, TCH)]
                            t1 = ropetmp.tile([64, TCH], F32, tag="rt1")
                            t2 = ropetmp.tile([64, TCH], F32, tag="rt2")
                            nc.vector.tensor_mul(t1, x1, co)
                            nc.vector.tensor_mul(t2, x2, si)
                            nc.vector.tensor_sub(dst[0:64, ts(c, TCH)], t1, t2)
                            t3 = ropetmp.tile([64, TCH], F32, tag="rt1")
                            t4 = ropetmp.tile([64, TCH], F32, tag="rt2")
                            nc.vector.tensor_mul(t3, x2, co)
                            nc.vector.tensor_mul(t4, x1, si)
                            nc.vector.tensor_add(dst[64:128, ts(c, TCH)], t3, t4)
                if j == 5:
                    # transpose v to natural [s, d] layout
                    for sb in range(NSB):
                        tp = tp_ps.tile([128, 128], BF16, tag="tp")
                        nc.tensor.transpose(tp, vT_t[:, ts(sb, 128)], ident_t)
                        nc.scalar.copy(out=vnat_t[:, sb, :], in_=tp)

        # ================= phase 2: attention =================
        p23per = ctx.enter_context(tc.tile_pool(name="p23per", bufs=1))
        attnT_t = p23per.tile([128, NHL, T], BF16, tag="attnT")
        mask_t = p23per.tile([128, NTC, TCH], BF16, tag="mask")
        wo_sb = p23per.tile([128, NHL, HID], BF16, tag="wo")
        nc.sync.dma_start(out=mask_t, in_=masks.ap())
        nc.sync.dma_start(out=wo_sb, in_=wo.ap())

        with ExitStack() as p2:
            ppool = p2.enter_context(tc.tile_pool(name="ppool", bufs=4))
            small = p2.enter_context(tc.tile_pool(name="small", bufs=2))
            sc_ps = p2.enter_context(tc.tile_pool(name="sc_ps", bufs=3, space="PSUM"))
            att_ps_pool = p2.enter_context(tc.tile_pool(name="att_ps", bufs=2, space="PSUM"))
            den_ps_pool = p2.enter_context(tc.tile_pool(name="den_ps", bufs=2, space="PSUM"))

            for h in range(NHL):
                for c in range(NTC):
                    nblk = 4 * (c + 1)
                    att_ps = att_ps_pool.tile([128, TCH], F32, tag="att")
                    den_ps = den_ps_pool.tile([1, TCH], F32, tag="den")
                    for sb in range(nblk):
                        scp = sc_ps.tile([128, TCH], F32, tag="sc")
                        nc.tensor.matmul(
                            scp, kT_t[:, ts(sb, 128)], qT_t[:, h, ts(c, TCH)],
                            start=True, stop=True,
                        )
                        p_sb = ppool.tile([128, TCH], BF16, tag="p")
                        nc.scalar.activation(p_sb, scp, AF.Exp, scale=SCALING)
                        r = sb - 4 * c
                        if r >= 0:
                            nc.vector.tensor_mul(p_sb, p_sb, mask_t[:, r, :])
                        nc.tensor.matmul(
                            att_ps, vnat_t[:, sb, :], p_sb,
                            start=(sb == 0), stop=(sb == nblk - 1),
                        )
                        nc.tensor.matmul(
                            den_ps, ones_s, p_sb,
                            start=(sb == 0), stop=(sb == nblk - 1),
                        )
                    rden = small.tile([1, TCH], F32, tag="rden")
                    nc.vector.reciprocal(rden, den_ps)
                    rep_ps = sc_ps.tile([128, TCH], F32, tag="sc")
                    nc.tensor.matmul(rep_ps, ones_r, rden, start=True, stop=True)
                    rep_sb = small.tile([128, TCH], F32, tag="rep")
                    nc.scalar.copy(out=rep_sb, in_=rep_ps)
                    nc.vector.tensor_mul(
                        attnT_t[:, h, ts(c, TCH)], att_ps, rep_sb
                    )

        # ================= phase 3: output projection =================
        opool = ctx.enter_context(tc.tile_pool(name="opool", bufs=3))
        out_ps_pool = ctx.enter_context(tc.tile_pool(name="out_ps", bufs=2, space="PSUM"))
        for ot in range(HID // 128):
            for half in range(2):
                ps2 = out_ps_pool.tile([128, 2, TCH], F32, tag="ops")
                for df in range(NHL):
                    for t2 in range(2):
                        cc = half * 2 + t2
                        nc.tensor.matmul(
                            ps2[:, t2, :], wo_sb[:, df, ts(ot, 128)],
                            attnT_t[:, df, ts(cc, TCH)],
                            start=(df == 0), stop=(df == NHL - 1),
                        )
                o_sb = opool.tile([128, 2 * TCH], F32, tag="osb")
                nc.vector.tensor_copy(o_sb, ps2.rearrange("p a b -> p (a b)"))
                nc.sync.dma_start(
                    out=outT.ap()[ts(ot, 128), ts(half, 2 * TCH)], in_=o_sb
                )

    nc.compile()
    return nc


_CACHE = {}


def _get_nc():
    if "nc" not in _CACHE:
        _CACHE["nc"] = build_kernel()
    return _CACHE["nc"]


def make_inputs(positions, hidden_states, Wqkv, Wo):
    """Host-side shard prep. Returns per-core input maps."""
    bf = ml_dtypes.bfloat16
    positions = np.asarray(positions)
    hidden_states = np.asarray(hidden_states, dtype=np.float32)
    Wqkv = np.asarray(Wqkv, dtype=np.float32)
    Wo = np.asarray(Wo, dtype=np.float32)

    hT = np.ascontiguousarray(hidden_states.astype(bf).T)  # [HID, T]

    half = D // 2
    inv_freq = (
        1.0 / (np.float32(ROPE_THETA) ** (np.arange(0, half, dtype=np.float32) / np.float32(half)))
    ).astype(np.float32)
    freqs = positions.astype(np.float32)[:, None] * inv_freq[None, :]  # [T, 64]
    cosT = np.ascontiguousarray(np.cos(freqs).astype(np.float32).T)
    sinT = np.ascontiguousarray(np.sin(freqs).astype(np.float32).T)

    p_ = np.arange(128)[:, None]
    j_ = np.arange(TCH)[None, :]
    masks = np.stack(
        [(j_ >= (128 * r + p_)) for r in range(NTC)], axis=1
    ).astype(bf)  # [128, 4, 512]

    q_size = NH * D
    kv_off = q_size + NKV * D
    in_maps = []
    for c in range(NCORES):
        qcols = Wqkv[:, 512 * c: 512 * (c + 1)]
        kcol = Wqkv[:, q_size + 128 * c: q_size + 128 * (c + 1)]
        vcol = Wqkv[:, kv_off + 128 * c: kv_off + 128 * (c + 1)]
        shard = np.concatenate([qcols, kcol, vcol], axis=1).astype(bf)  # [HID, 768]
        wq_dev = np.ascontiguousarray(
            shard.reshape(HO, 128, 6, 128).transpose(2, 1, 0, 3)
        )  # [6, 128, HO, 128]
        wo_shard = Wo[512 * c: 512 * (c + 1), :].astype(bf)  # [512, HID]
        wo_dev = np.ascontiguousarray(
            wo_shard.reshape(NHL, 128, HID).transpose(1, 0, 2)
        )  # [128, NHL, HID]
        in_maps.append(
            {
                "hT": hT,
                "wqkv": wq_dev,
                "wo": wo_dev,
                "cosT": cosT,
                "sinT": sinT,
                "masks": masks,
            }
        )
    return in_maps


def kernel(positions, hidden_states, Wqkv, Wo):
    in_maps = make_inputs(positions, hidden_states, Wqkv, Wo)
    res = run_bass_kernel_spmd(_get_nc(), in_maps, list(range(NCORES)))
    acc = res.results[0]["outT"].copy()
    for c in range(1, NCORES):
        acc += res.results[c]["outT"]
    return np.ascontiguousarray(acc.T)
